# revision 1
# baseline (speedup 1.0000x reference)
"""Trainium2 Bass kernel for uniform cubic B-spline basis (Cox-de Boor, degree 3).

Math: knots = linspace(-pi, pi, 256) are uniform, so all 252 basis functions are
shifts of one cardinal cubic C(s) supported on [0,4):
    C(s) = (1/6) * sum_k (-1)^k binom(4,k) relu(s-k)^3,  s clamped to [0,4].
For x in knot interval i (u = (x+pi)/h, i = floor(u)), the output row is zero
except columns cstart..cstart+3 with cstart = clip(i-3, 0, 248), and
    row[cstart+m] = C(u - cstart - m).
This handles the edge rows (i<3, i>251) exactly too, because the knot vector is
unclamped uniform.

Per core (131072 rows): rows r = p*1024 + f live in a [128 partitions, 1024]
compute domain. DVE computes u, cstart, flat output element offsets
(r*252 + cstart, int32) and the 4 payload values per row (interleaved
[128, 4*F]). The runtime pre-zeros ExternalOutput buffers (run_bass_kernel_spmd
contract: the native path pre-zeros them, the PJRT path donates zero buffers),
so only the 4 nonzero values per row are written: GPSIMD indirect DMA scatters
16 B per row (one instruction per 128 rows - HW consumes one offset per
partition per instruction, writing each partition's contiguous in_ chunk).
"""
import sys
import types

sys.path.insert(0, "/opt/trn_rl_repo")
sys.path.insert(0, "/root/.axon_site/_ro/trn_rl_repo")

import numpy as np


def _ensure_axon_hooks():
    # antenv.axon_hooks is absent in this image; shim it so trace=True works.
    if "antenv.axon_hooks" in sys.modules:
        return
    try:
        import antenv
    except ImportError:
        return
    m = types.ModuleType("antenv.axon_hooks")
    m._hook = None
    m.set_axon_ntff_profile_hook = lambda h: setattr(m, "_hook", h)
    m.get_axon_ntff_profile_hook = lambda: m._hook
    sys.modules["antenv.axon_hooks"] = m
    antenv.axon_hooks = m
    try:
        from trn_agent_boot.trn_boot import _ntff_profile_via_ctypes

        hook = _ntff_profile_via_ctypes("/opt/axon/libaxon_pjrt.so")
        if hook is not None:
            m.set_axon_ntff_profile_hook(hook)
    except Exception:
        pass


_ensure_axon_hooks()

import concourse.bass as bass
import concourse.mybir as mybir

N = 1_048_576
NCORES = 8
PC = N // NCORES          # rows per core = 131072
P = 128
FTOT = PC // P            # 1024 rows per partition
COLS = 252
NUM_KNOTS = 256

PI = float(np.float32(np.pi))
H = float(np.float32(2.0 * np.pi / (NUM_KNOTS - 1)))
INVH = float(np.float32(1.0) / np.float32(H))
# floor(u) via round-to-nearest(u - 0.5): fold the -0.5 into the bias constant
C1 = float(np.float32(PI - 0.5 * H))

AOT = mybir.AluOpType
F32 = mybir.dt.float32
I32 = mybir.dt.int32

# C(s) = sum_k COEF6[k] * relu(s-k)^3 with the 1/6 folded in
COEF6 = [1.0 / 6.0, -4.0 / 6.0, 1.0, -4.0 / 6.0, 1.0 / 6.0]

# Staggered f-chunk sizes: small first chunks so the GPSIMD scatter (the
# bottleneck, ~1.55 us per 128-row instruction) starts as early as possible.
CHUNKS = [128, 128, 256, 512]
FCMAX = max(CHUNKS)


def build_nc():
    nc = bass.Bass()
    x_in = nc.declare_dram_parameter("x", [PC, 1], F32, isOutput=False)
    out = nc.declare_dram_parameter("out", [PC, COLS], F32, isOutput=True)

    x_flat = x_in[:, 0].rearrange("(p f) -> p f", p=P)  # [128, 1024]
    nchunks = len(CHUNKS)
    starts = [sum(CHUNKS[:i]) for i in range(nchunks)]
    assert sum(CHUNKS) == FTOT

    iota_base = nc.alloc_sbuf_tensor("iota_base", [P, FTOT], I32)
    mneg_i = nc.alloc_sbuf_tensor("mneg_i", [P, 4 * FCMAX], I32)
    mneg = nc.alloc_sbuf_tensor("mneg", [P, 4 * FCMAX], F32)

    with (
        nc.semaphore("isem") as isem,   # iota consts ready
        nc.semaphore("xsem") as xsem,   # x chunk loaded
        nc.semaphore("csem") as csem,   # chunk compute done
        nc.semaphore("dsem") as dsem,   # scatter DMA completions
    ):
        xt = [nc.alloc_sbuf_tensor(f"xt{c}", [P, fc], F32) for c, fc in enumerate(CHUNKS)]
        u = [nc.alloc_sbuf_tensor(f"u{c}", [P, fc], F32) for c, fc in enumerate(CHUNKS)]
        us = [nc.alloc_sbuf_tensor(f"us{c}", [P, fc], F32) for c, fc in enumerate(CHUNKS)]
        ci = nc.alloc_sbuf_tensor("ci", [P, FTOT], I32)
        cst = [nc.alloc_sbuf_tensor(f"cst{c}", [P, fc], F32) for c, fc in enumerate(CHUNKS)]
        d = [nc.alloc_sbuf_tensor(f"d{c}", [P, fc], F32) for c, fc in enumerate(CHUNKS)]
        offi = nc.alloc_sbuf_tensor("offi", [P, FTOT], I32)
        v = [nc.alloc_sbuf_tensor(f"v{c}", [P, 4 * fc], F32) for c, fc in enumerate(CHUNKS)]
        r = [nc.alloc_sbuf_tensor(f"r{c}", [P, 4 * fc], F32) for c, fc in enumerate(CHUNKS)]
        r2 = [nc.alloc_sbuf_tensor(f"r2{c}", [P, 4 * fc], F32) for c, fc in enumerate(CHUNKS)]
        t = [nc.alloc_sbuf_tensor(f"t{c}", [P, 4 * fc], F32) for c, fc in enumerate(CHUNKS)]
        acc = [nc.alloc_sbuf_tensor(f"acc{c}", [P, 4 * fc], F32) for c, fc in enumerate(CHUNKS)]

        with nc.Block() as block:

            @block.sync
            def _(s: bass.BassEngine):
                for ch, fc in enumerate(CHUNKS):
                    s.dma_start(
                        out=xt[ch][:], in_=x_flat[:, starts[ch]: starts[ch] + fc]
                    ).then_inc(xsem, 16)

            @block.gpsimd
            def _(g: bass.BassEngine):
                g.iota(
                    iota_base[:], pattern=[[COLS, FTOT]], base=0,
                    channel_multiplier=FTOT * COLS,
                )
                g.iota(
                    mneg_i[:], pattern=[[0, FCMAX], [-1, 4]], base=0,
                    channel_multiplier=0,
                ).then_inc(isem, 1)
                # int32 adds on GPSIMD: DVE's fp32 ALU would round flat
                # offsets above 2^24 to even, shifting scatters by 1. Each
                # gpsimd tensor op costs ~20us to launch, so do only two:
                # one for chunk 0 (so scatters start early), one for the rest.
                f0 = CHUNKS[0]
                # warm up the Q7 tensor-op ucode (first tensor op after boot
                # costs ~77us; subsequent ones ~2us) while DVE still computes
                g.tensor_tensor(
                    out=offi[:, :2], in0=iota_base[:, :2],
                    in1=iota_base[:, :2], op=AOT.add,
                )
                g.wait_ge(csem, 1)
                g.tensor_tensor(
                    out=offi[:, :f0], in0=iota_base[:, :f0],
                    in1=ci[:, :f0], op=AOT.add,
                )
                for ch, fc in enumerate(CHUNKS):
                    if ch == 1:
                        g.wait_ge(csem, len(CHUNKS))
                        g.tensor_tensor(
                            out=offi[:, f0:], in0=iota_base[:, f0:],
                            in1=ci[:, f0:], op=AOT.add,
                        )
                    for f in range(starts[ch], starts[ch] + fc):
                        g.indirect_dma_start(
                            out=out[:, :],
                            out_offset=bass.IndirectOffsetOnAxis(
                                ap=offi[:, f: f + 1], axis=1
                            ),
                            in_=acc[ch][:, 4 * (f - starts[ch]): 4 * (f - starts[ch]) + 4],
                            in_offset=None,
                        ).then_inc(dsem, 16)
                g.wait_ge(dsem, 16 * FTOT)

            @block.vector
            def _(ve: bass.BassEngine):
                ve.wait_ge(isem, 1)
                ve.tensor_copy(out=mneg[:], in_=mneg_i[:])
                for ch, fc in enumerate(CHUNKS):
                    ve.wait_ge(xsem, 16 * (ch + 1))
                    # u (unshifted) and floor(u) via shifted round-to-nearest
                    ve.tensor_scalar(
                        out=u[ch][:], in0=xt[ch][:], scalar1=PI, scalar2=INVH,
                        op0=AOT.add, op1=AOT.mult,
                    )
                    ve.tensor_scalar(
                        out=us[ch][:], in0=xt[ch][:], scalar1=C1, scalar2=INVH,
                        op0=AOT.add, op1=AOT.mult,
                    )
                    cisl = ci[:, starts[ch]: starts[ch] + fc]
                    ve.tensor_copy(out=cisl, in_=us[ch][:])  # rint = floor(u)
                    ve.tensor_copy(out=cst[ch][:], in_=cisl)
                    # cstart = clip(i-3, 0, 248)
                    ve.tensor_scalar(
                        out=cst[ch][:], in0=cst[ch][:], scalar1=3.0, scalar2=0.0,
                        op0=AOT.subtract, op1=AOT.max,
                    )
                    ve.tensor_scalar_min(
                        out=cst[ch][:], in0=cst[ch][:], scalar1=248.0
                    )
                    # d = u - cstart
                    ve.tensor_tensor(
                        out=d[ch][:], in0=u[ch][:], in1=cst[ch][:],
                        op=AOT.subtract,
                    )
                    # payload v[p, 4f+m] = d - m, clamped at 4
                    ve.tensor_tensor(
                        out=v[ch][:].rearrange("p (f m) -> p f m", m=4),
                        in0=d[ch][:].unsqueeze(2).broadcast_to([P, fc, 4]),
                        in1=mneg[:, : 4 * fc].rearrange("p (f m) -> p f m", m=4),
                        op=AOT.add,
                    )
                    ve.tensor_scalar_min(out=v[ch][:], in0=v[ch][:], scalar1=4.0)
                    for k in range(5):
                        ve.tensor_scalar(
                            out=r[ch][:], in0=v[ch][:], scalar1=float(k),
                            scalar2=0.0, op0=AOT.subtract, op1=AOT.max,
                        )
                        ve.tensor_tensor(
                            out=r2[ch][:], in0=r[ch][:], in1=r[ch][:],
                            op=AOT.mult,
                        )
                        dst = acc[ch] if k == 0 else t[ch]
                        ve.scalar_tensor_tensor(
                            out=dst[:], in0=r2[ch][:], scalar=COEF6[k],
                            in1=r[ch][:], op0=AOT.mult, op1=AOT.mult,
                        )
                        if k > 0:
                            ve.tensor_tensor(
                                out=acc[ch][:], in0=acc[ch][:], in1=t[ch][:],
                                op=AOT.add,
                            )
                    # cstart as int32, last so its completion implies acc is
                    # also final (the offset add happens on GPSIMD)
                    ve.tensor_copy(
                        out=ci[:, starts[ch]: starts[ch] + fc], in_=cst[ch][:]
                    ).then_inc(csem, 1)

    return nc


_CACHED = {}


def kernel(**inputs) -> np.ndarray:
    from concourse.bass_utils import run_bass_kernel_spmd

    x = np.asarray(inputs["x"], dtype=np.float32).reshape(N, 1)
    if "nc" not in _CACHED:
        _CACHED["nc"] = build_nc()
    nc = _CACHED["nc"]
    in_maps = [{"x": x[c * PC: (c + 1) * PC]} for c in range(NCORES)]
    res = run_bass_kernel_spmd(nc, in_maps, list(range(NCORES)))
    return np.concatenate([r["out"] for r in res.results], axis=0)


if __name__ == "__main__":
    rng = np.random.default_rng(0)
    xs = rng.uniform(-np.pi, np.pi, size=(N, 1)).astype(np.float32)
    o = kernel(x=xs)
    print("out", o.shape, o.dtype, float(np.abs(o).max()))

